# revision 27
# baseline (speedup 1.0000x reference)
"""DGCNN Trainium kernel v2: per-core HALF batch (4 batches x 2 halves = 8 cores).

Per core (NI=2048 query points, full N=4096 candidate set):
  S: bf16 PD scores via PE (4x [4,128]x[4,1024] per block) -> PSUM; DVE
     16-wide segment max; top-32 segments (max8 rounds); ONE batched
     indirect-DMA candidate gather; exact fp32 rescore (Pool); top-20;
     decode via onehot dot; ONE batched indirect-DMA point gather straight
     into the feature tile (padded 32-float point rows).
  F: 19-channel feature math, n-major, two scratch lanes (DVE / Pool) to
     keep both engines busy; ACT ops batched per function (Sqrt /
     Reciprocal / Arctan) to avoid table reloads; PE transpose to
     channel-major bf16 fcm.
  M: 3-layer MLP in bf16 (1024-col moving operands). L1/L2: matmul once,
     stats via ACT-accumulated sum/sumsq during PSUM->SBUF copy, then
     normalize+relu from stored preacts. L3: bn_stats pass + recompute
     pass with fused norm+relu, k-max split across DVE and Pool.
  GroupNorm stats are per-core (half batch) -- validated ~0.7% rel err.
"""
import numpy as np
import ml_dtypes
from contextlib import ExitStack

import concourse.bass as bass
import concourse.tile as tile
from concourse import mybir

dt = mybir.dt
F32, U32, I32, BF16 = dt.float32, dt.uint32, dt.int32, dt.bfloat16
AF = mybir.ActivationFunctionType
OP = mybir.AluOpType
AX = mybir.AxisListType

N = 4096
NBLK = 16                 # blocks per core (half batch)
NI = NBLK * 128           # 2048
K = 20
SEGW = 16
NSEG = 256
NCS = 20                  # candidate segments (exact scores)
NCAND = NCS * SEGW        # 512
NKK_G = 5                 # k-groups of 4
CW = 512                  # matmul moving width (PSUM bank limit: 512 f32)
NCH = NI // CW            # 4
NU = NCH * NKK_G * 2      # 40 units per layer (L1/L2)
NU3 = NU * 2              # 80 units (L3, kx split)
GN_EPS = 1e-5
PIH = 1.5707963267948966
HW = NBLK * K             # 320 free elems for feature ops
ANGLE_CH = (9, 10, 11, 16, 17, 18)

BF = ml_dtypes.bfloat16


def host_prep(data_b, half, W1, g1, b1, W2, g2, b2, W3, g3, b3):
    """Per-core host tables. data_b: (6, N) f32; half in {0,1}."""
    x = data_b[:3].astype(np.float32)
    nrm = data_b[3:6].astype(np.float32)
    xx = ((x[0] * x[0] + x[1] * x[1]) + x[2] * x[2]).astype(np.float32)
    lo, hi = half * NI, (half + 1) * NI
    q4 = np.stack([2 * x[0, lo:hi], 2 * x[1, lo:hi], 2 * x[2, lo:hi],
                   np.ones(NI, np.float32)])
    r4 = np.stack([x[0], x[1], x[2], -xx])
    qtab = q4.astype(BF)
    rtab = r4.astype(BF)
    dqtab = (q4 - qtab.astype(np.float32)).astype(BF)
    drtab = (r4 - rtab.astype(np.float32)).astype(BF)
    seg = np.zeros((NSEG, SEGW, 4), np.float32)
    seg[:, :, 0] = x[0].reshape(NSEG, SEGW)
    seg[:, :, 1] = x[1].reshape(NSEG, SEGW)
    seg[:, :, 2] = x[2].reshape(NSEG, SEGW)
    seg[:, :, 3] = xx.reshape(NSEG, SEGW)
    seg_tab = seg.reshape(NSEG, 64)
    pt = np.zeros((N, 32), np.float32)
    pt[:, 0:3] = x.T          # -> ch0-2 (gxyz)
    pt[:, 6:9] = x.T          # -> ch6-8 (lxyz after subtracting xc)
    pt[:, 19:22] = nrm.T      # -> scratch ch19-21 (neighbor normal)
    ctr = x.T[lo:hi].reshape(NBLK, 128, 3).transpose(1, 0, 2)
    ctr2 = np.ascontiguousarray(2.0 * ctr).astype(np.float32)
    cnrm = np.ascontiguousarray(
        nrm.T[lo:hi].reshape(NBLK, 128, 3).transpose(1, 0, 2))
    iota = np.broadcast_to(np.arange(NCS, dtype=np.float32), (128, NCS)).copy()
    idn = np.eye(128, dtype=np.float32)
    # angle channels carry a folded-out factor 2 (atan half-angle): fold into W1
    W1e = W1.copy().astype(np.float32)
    for c in ANGLE_CH:
        W1e[:, c] = 2.0 * W1e[:, c]
    W1p = np.zeros((32, 64), np.float32)
    W1p[:19, :] = W1e.T
    w1a = np.zeros((128, 128), np.float32)
    w1b = np.zeros((128, 128), np.float32)
    for kk in range(2):
        w1a[kk * 32:(kk + 1) * 32, kk * 64:(kk + 1) * 64] = W1p
        w1b[(kk + 2) * 32:(kk + 3) * 32, kk * 64:(kk + 1) * 64] = W1p
    w2bd = np.zeros((128, 128), np.float32)
    w2bd[:64, :64] = W2.T
    w2bd[64:, 64:] = W2.T
    w3t = np.ascontiguousarray(np.vstack([W3.T, W3.T]))  # [128, 96]
    m1_12 = np.zeros((128, 16), np.float32)
    for p in range(128):
        m1_12[p, (p % 64) // 4] = 1.0
    e_12 = np.zeros((16, 128), np.float32)
    for p in range(128):
        e_12[(p % 64) // 4, p] = 1.0
    m1_3 = np.zeros((96, 16), np.float32)
    for p in range(96):
        m1_3[p, p // 6] = 1.0
    e_3 = np.zeros((16, 96), np.float32)
    for p in range(96):
        e_3[p // 6, p] = 1.0
    return {
        "qtab": qtab, "rtab": rtab, "dqtab": dqtab, "drtab": drtab, "seg_tab": seg_tab, "pt_tab": pt,
        "ctr2": ctr2, "cnrm": cnrm, "iota": iota, "idn": idn,
        "w1a": w1a.astype(BF), "w1b": w1b.astype(BF),
        "w2bd": w2bd.astype(BF), "w3t": w3t.astype(BF),
        "m1_12": m1_12, "e_12": e_12, "m1_3": m1_3, "e_3": e_3,
        "g1rep": np.tile(g1, 2).reshape(128, 1).astype(np.float32),
        "b1rep": np.tile(b1, 2).reshape(128, 1).astype(np.float32),
        "g2rep": np.tile(g2, 2).reshape(128, 1).astype(np.float32),
        "b2rep": np.tile(b2, 2).reshape(128, 1).astype(np.float32),
        "g3rep": g3.reshape(96, 1).astype(np.float32),
        "b3rep": b3.reshape(96, 1).astype(np.float32),
    }


INPUT_SPECS = {
    "qtab": ((4, NI), BF16), "rtab": ((4, N), BF16),
    "dqtab": ((4, NI), BF16), "drtab": ((4, N), BF16),
    "seg_tab": ((NSEG, 64), F32), "pt_tab": ((N, 32), F32),
    "ctr2": ((128, NBLK, 3), F32), "cnrm": ((128, NBLK, 3), F32),
    "iota": ((128, NCS), F32), "idn": ((128, 128), F32),
    "w1a": ((128, 128), BF16), "w1b": ((128, 128), BF16),
    "w2bd": ((128, 128), BF16), "w3t": ((128, 96), BF16),
    "m1_12": ((128, 16), F32), "e_12": ((16, 128), F32),
    "m1_3": ((96, 16), F32), "e_3": ((16, 96), F32),
    "g1rep": ((128, 1), F32), "b1rep": ((128, 1), F32),
    "g2rep": ((128, 1), F32), "b2rep": ((128, 1), F32),
    "g3rep": ((96, 1), F32), "b3rep": ((96, 1), F32),
}


def declare_inputs(nc):
    return {k: nc.dram_tensor(k, list(sh), dty, kind="ExternalInput").ap()
            for k, (sh, dty) in INPUT_SPECS.items()}


import os
DBG = os.environ.get("KDBG", "")


def build(nc, tc, ctx, din, out_ap):
    consts = ctx.enter_context(tc.tile_pool(name="consts", bufs=1))
    fcmp = ctx.enter_context(tc.tile_pool(name="fcm", bufs=1))

    ld = {}
    for name in ["qtab", "rtab", "dqtab", "drtab", "ctr2", "cnrm", "iota", "idn", "w1a", "w1b",
                 "w2bd", "w3t", "m1_12", "e_12", "m1_3", "e_3",
                 "g1rep", "b1rep", "g2rep", "b2rep", "g3rep", "b3rep"]:
        t = consts.tile(list(INPUT_SPECS[name][0]), INPUT_SPECS[name][1],
                        tag=name)
        nc.gpsimd.dma_start(t[:], din[name][:])
        ld[name] = t

    fcm = fcmp.tile([128, NKK_G, NI], BF16)

    stp = ctx.enter_context(tc.tile_pool(name="stats", bufs=1))
    psmm = ctx.enter_context(tc.tile_pool(name="psmm", bufs=2, space="PSUM"))
    prep = ctx.enter_context(tc.tile_pool(name="prep", bufs=1))
    sqp = ctx.enter_context(tc.tile_pool(name="sqp", bufs=2))
    s1s = stp.tile([128, NU], F32, tag="s1s")
    s1q = stp.tile([128, NU], F32, tag="s1q")
    pre = prep.tile([128, NU, CW], BF16, tag="pre")

    sfctx = ExitStack()
    fp = sfctx.enter_context(tc.tile_pool(name="feat", bufs=1))
    F = fp.tile([128, NBLK, K, 32], F32)
    sp = sfctx.enter_context(tc.tile_pool(name="fscr", bufs=1))
    pstp = sfctx.enter_context(tc.tile_pool(name="pstp", bufs=2,
                                            space="PSUM"))

    # ================= Phase S =================
    with ExitStack() as sctx:
        selp = sctx.enter_context(tc.tile_pool(name="sel", bufs=4))
        pspd = sctx.enter_context(tc.tile_pool(name="pspd", bufs=4,
                                               space="PSUM"))
        pend = []
        for blk in range(NBLK):
            segmax = selp.tile([128, NSEG], F32, tag="segmax")
            for q in range(8):
                pd = pspd.tile([128, 32, SEGW], F32, tag="pd")
                pdf = pd[:].rearrange("p s w -> p (s w)")
                qsl = slice(blk * 128, (blk + 1) * 128)
                rsl = slice(q * 512, (q + 1) * 512)
                nc.tensor.matmul(pdf, ld["qtab"][:, qsl], ld["rtab"][:, rsl],
                                 start=True, stop=False)
                nc.tensor.matmul(pdf, ld["qtab"][:, qsl], ld["drtab"][:, rsl],
                                 start=False, stop=False)
                nc.tensor.matmul(pdf, ld["dqtab"][:, qsl], ld["rtab"][:, rsl],
                                 start=False, stop=True)
                nc.vector.tensor_reduce(
                    segmax[:, q * 32:(q + 1) * 32], pd[:], AX.X, OP.max)

            segv = selp.tile([128, 24], F32, tag="segv")
            segi = selp.tile([128, 24], U32, tag="segi")
            for r in range(3):
                nc.vector.max(segv[:, r * 8:(r + 1) * 8], segmax[:])
                nc.vector.max_index(segi[:, r * 8:(r + 1) * 8],
                                    segv[:, r * 8:(r + 1) * 8], segmax[:])
                if r < 2:
                    nc.vector.match_replace(segmax[:],
                                            segv[:, r * 8:(r + 1) * 8],
                                            segmax[:], -1e30)
            segi32 = selp.tile([128, NCS], I32, tag="segi32")
            nc.vector.tensor_copy(segi32[:], segi[:, :NCS])
            segf = selp.tile([128, NCS], F32, tag="segf")
            nc.vector.tensor_copy(segf[:], segi[:, :NCS])

            cand = selp.tile([128, NCS, 64], F32, tag="cand")
            for c in range(NCS):
                nc.gpsimd.indirect_dma_start(
                    out=cand[:, c, :], out_offset=None, in_=din["seg_tab"][:],
                    in_offset=bass.IndirectOffsetOnAxis(
                        ap=segi32[:, c:c + 1], axis=0))

            cxyz = cand[:].rearrange("p c (m d) -> p (c m) d", d=4)
            sc = selp.tile([128, NCAND], F32, tag="sc")
            q0 = ld["ctr2"][:, blk, 0:1]
            q1 = ld["ctr2"][:, blk, 1:2]
            q2 = ld["ctr2"][:, blk, 2:3]
            nc.vector.tensor_scalar(sc[:], cxyz[:, :, 0], q0, None, OP.mult)
            nc.vector.scalar_tensor_tensor(sc[:], cxyz[:, :, 1], q1, sc[:],
                                           OP.mult, OP.add)
            nc.vector.scalar_tensor_tensor(sc[:], cxyz[:, :, 2], q2, sc[:],
                                           OP.mult, OP.add)
            nc.vector.scalar_tensor_tensor(sc[:], cxyz[:, :, 3], -1.0, sc[:],
                                           OP.mult, OP.add)

            canv = selp.tile([128, 24], F32, tag="canv")
            cani = selp.tile([128, 24], U32, tag="cani")
            for r in range(3):
                nc.vector.max(canv[:, r * 8:(r + 1) * 8], sc[:])
                nc.vector.max_index(cani[:, r * 8:(r + 1) * 8],
                                    canv[:, r * 8:(r + 1) * 8], sc[:])
                if r < 2:
                    nc.vector.match_replace(sc[:], canv[:, r * 8:(r + 1) * 8],
                                            sc[:], -1e30)

            slot = selp.tile([128, K], U32, tag="slot")
            memb = selp.tile([128, K], U32, tag="memb")
            nc.vector.tensor_scalar(slot[:], cani[:, :K], 4, None,
                                    OP.logical_shift_right)
            nc.vector.tensor_scalar(memb[:], cani[:, :K], 15, None,
                                    OP.bitwise_and)
            slotf = selp.tile([128, K], F32, tag="slotf")
            membf = selp.tile([128, K], F32, tag="membf")
            nc.vector.tensor_copy(slotf[:], slot[:])
            nc.vector.tensor_copy(membf[:], memb[:])

            eng = nc.vector
            eq = selp.tile([128, K, NCS], F32, tag="eq")
            eng.tensor_tensor(
                eq[:],
                slotf[:].rearrange("p r -> p r ()").broadcast_to(
                    [128, K, NCS]),
                ld["iota"][:].rearrange("p c -> p () c").broadcast_to(
                    [128, K, NCS]),
                OP.is_equal)
            eng.tensor_tensor(
                eq[:], eq[:],
                segf[:].rearrange("p c -> p () c").broadcast_to([128, K, NCS]),
                OP.mult)
            segsel = selp.tile([128, K], F32, tag="segsel")
            nc.vector.tensor_reduce(segsel[:], eq[:], AX.X, OP.add)
            nbrf = selp.tile([128, K], F32, tag="nbrf")
            nc.vector.scalar_tensor_tensor(nbrf[:], segsel[:], 16.0,
                                           membf[:], OP.mult, OP.add)
            nbri = selp.tile([128, K], I32, tag="nbri")
            nc.vector.tensor_copy(nbri[:], nbrf[:])

            pend.append((blk, nbri))
            if len(pend) > 1:
                pblk, pnbri = pend.pop(0)
                for kk in range(K):
                    nc.gpsimd.indirect_dma_start(
                        out=F[:, pblk, kk, :], out_offset=None,
                        in_=din["pt_tab"][:],
                        in_offset=bass.IndirectOffsetOnAxis(
                            ap=pnbri[:, kk:kk + 1], axis=0))
        for pblk, pnbri in pend:
            for kk in range(K):
                nc.gpsimd.indirect_dma_start(
                    out=F[:, pblk, kk, :], out_offset=None,
                    in_=din["pt_tab"][:],
                    in_offset=bass.IndirectOffsetOnAxis(
                        ap=pnbri[:, kk:kk + 1], axis=0))
            if DBG == "nbr" and blk == 0:
                nc.gpsimd.dma_start(out_ap[:, :K], nbrf[:96, :])
            if DBG == "segs" and blk == 0:
                nc.gpsimd.dma_start(out_ap[:, :NCS], segf[:96, :])
            if DBG == "sc" and blk == 0:
                nc.gpsimd.dma_start(out_ap[:, :NCAND], sc[:96, :])

        if DBG == "gath":
            nc.gpsimd.dma_start(
                out_ap[:, :640],
                F[:96, 0].rearrange("p k c -> p (k c)"))
        if DBG in ("gath", "nbr", "segs", "sc"):
            return

    # ====== Phase F (two halves of 8 blocks) + L1 interleaved under S ======
    with ExitStack() as fctx:
        EN = [nc.vector, nc.vector]
        NBH = 8
        HWH = NBH * K

        for fh in range(2):
            b0, b1 = fh * NBH, (fh + 1) * NBH

            def ch(c):
                return F[:, b0:b1, :, c]

            def cb(t, d):
                return t[:, b0:b1, d:d + 1].broadcast_to([128, NBH, K])

            sa = [sp.tile([128, HWH], F32, name=f"sa{i}", tag=f"sa{i}")
                  for i in range(2)]
            sb = [sp.tile([128, HWH], F32, name=f"sb{i}", tag=f"sb{i}")
                  for i in range(2)]
            scx = [sp.tile([128, HWH], F32, name=f"scx{i}", tag=f"scx{i}")
                   for i in range(2)]
            scy = [sp.tile([128, HWH], F32, name=f"scy{i}", tag=f"scy{i}")
                   for i in range(2)]
            scz = [sp.tile([128, HWH], F32, name=f"scz{i}", tag=f"scz{i}")
                   for i in range(2)]
            d2 = sp.tile([128, HWH], F32, tag="d2")
            d2n = sp.tile([128, HWH], F32, tag="d2n")
            nc3 = sp.tile([128, 3, HWH], F32, tag="nc3")
            y2s = sp.tile([128, 4, HWH], F32, tag="y2s")
            dens = sp.tile([128, 6, HWH], F32, tag="dens")
            mns = sp.tile([128, 6, HWH], F32, tag="mns")
            mxs = sp.tile([128, 6, HWH], F32, tag="mxs")
            d16 = sp.tile([128, HWH], F32, tag="d16")
            cmt = sp.tile([128, 3, NBH], F32, tag="cmt")
            nrt = sp.tile([128, 3, NBH], F32, tag="nrt")
            nl2 = sp.tile([128, NBH], F32, tag="nl2")
            sm1 = sp.tile([128, NBH], F32, tag="sm1")

            def v3(t, i):
                return t[:, i, :].rearrange("p (b k) -> p b k", k=K)

            def fl(t):
                return t[:].rearrange("p (b k) -> p b k", k=K)

            for d in range(3):
                nc.vector.tensor_scalar(ch(3 + d), cb(ld["ctr2"], d), 0.5,
                                        None, OP.mult)
                nc.vector.tensor_tensor(ch(6 + d), ch(6 + d), ch(3 + d),
                                        OP.subtract)

            def sumsq3(E, lane, out, v0, v1v, v2v):
                E.tensor_tensor(out, v0, v0, OP.mult)
                E.tensor_tensor(fl(sa[lane]), v1v, v1v, OP.mult)
                E.tensor_tensor(out, out, fl(sa[lane]), OP.add)
                E.tensor_tensor(fl(sa[lane]), v2v, v2v, OP.mult)
                E.tensor_tensor(out, out, fl(sa[lane]), OP.add)

            sumsq3(nc.vector, 1, fl(d2), ch(6), ch(7), ch(8))
            for d in range(3):
                nc.vector.tensor_reduce(cmt[:, d, :], ch(d), AX.X, OP.add)
                nc.vector.tensor_scalar(cmt[:, d, :], cmt[:, d, :], 1.0 / K,
                                        None, OP.mult)
                nc.vector.scalar_tensor_tensor(nrt[:, d, :],
                                               ld["ctr2"][:, b0:b1, d], -0.5,
                                               cmt[:, d, :], OP.mult, OP.add)
            nc.vector.tensor_tensor(nl2[:], nrt[:, 0, :], nrt[:, 0, :],
                                    OP.mult)
            nc.vector.tensor_tensor(sm1[:], nrt[:, 1, :], nrt[:, 1, :],
                                    OP.mult)
            nc.vector.tensor_tensor(nl2[:], nl2[:], sm1[:], OP.add)
            nc.vector.tensor_tensor(sm1[:], nrt[:, 2, :], nrt[:, 2, :],
                                    OP.mult)
            nc.vector.tensor_tensor(nl2[:], nl2[:], sm1[:], OP.add)
            nc.scalar.activation(sm1[:], nl2[:], AF.Sqrt)
            nc.vector.tensor_scalar(
                ch(13), sm1[:].rearrange("p b -> p b ()").broadcast_to(
                    [128, NBH, K]), 1.0, None, OP.mult)
            for d in range(3):
                nc.vector.tensor_tensor(
                    v3(nc3, d), ch(6 + d),
                    nrt[:, d, :].rearrange("p b -> p b ()").broadcast_to(
                        [128, NBH, K]), OP.subtract)
            sumsq3(nc.vector, 1, fl(d2n), v3(nc3, 0), v3(nc3, 1), v3(nc3, 2))
            nc.scalar.activation(ch(12), fl(d2), AF.Sqrt)
            nc.scalar.activation(ch(14), fl(d2n), AF.Sqrt)
            nc.gpsimd.tensor_copy(ch(15), ch(12))

            def cross_dot(lane, v1, v2, y2ap, dotap):
                E = EN[lane]
                a, b = fl(sa[lane]), fl(sb[lane])
                cx, cy, cz = fl(scx[lane]), fl(scy[lane]), fl(scz[lane])
                E.tensor_tensor(a, v1[1], v2[2], OP.mult)
                E.tensor_tensor(b, v1[2], v2[1], OP.mult)
                E.tensor_tensor(cx, a, b, OP.subtract)
                E.tensor_tensor(a, v1[2], v2[0], OP.mult)
                E.tensor_tensor(b, v1[0], v2[2], OP.mult)
                E.tensor_tensor(cy, a, b, OP.subtract)
                E.tensor_tensor(a, v1[0], v2[1], OP.mult)
                E.tensor_tensor(b, v1[1], v2[0], OP.mult)
                E.tensor_tensor(cz, a, b, OP.subtract)
                E.tensor_tensor(y2ap, cx, cx, OP.mult)
                E.tensor_tensor(a, cy, cy, OP.mult)
                E.tensor_tensor(y2ap, y2ap, a, OP.add)
                E.tensor_tensor(a, cz, cz, OP.mult)
                E.tensor_tensor(y2ap, y2ap, a, OP.add)
                E.tensor_tensor(dotap, v1[0], v2[0], OP.mult)
                E.tensor_tensor(a, v1[1], v2[1], OP.mult)
                E.tensor_tensor(dotap, dotap, a, OP.add)
                E.tensor_tensor(a, v1[2], v2[2], OP.mult)
                E.tensor_tensor(dotap, dotap, a, OP.add)

            lv = (ch(6), ch(7), ch(8))
            nnv = (ch(19), ch(20), ch(21))
            cnv = tuple(cb(ld["cnrm"], d) for d in range(3))
            nrv = tuple(nrt[:, d, :].rearrange("p b -> p b ()").broadcast_to(
                [128, NBH, K]) for d in range(3))

            def dv(i):
                return dens[:, i, :].rearrange("p (b k) -> p b k", k=K)

            def yv(i):
                return y2s[:, i, :].rearrange("p (b k) -> p b k", k=K)

            def mv(i):
                return mns[:, i, :].rearrange("p (b k) -> p b k", k=K)

            def xv(i):
                return mxs[:, i, :].rearrange("p (b k) -> p b k", k=K)

            cross_dot(0, cnv, lv, yv(0), dv(0))
            nc.vector.tensor_tensor(dv(0), dv(0), ch(12), OP.add)
            cross_dot(1, nnv, lv, yv(1), dv(1))
            nc.vector.tensor_tensor(dv(1), dv(1), ch(12), OP.add)
            cross_dot(0, cnv, nnv, yv(2), dv(2))
            nc.vector.tensor_scalar(dv(2), dv(2), 1.0, None, OP.add)
            cross_dot(0, nrv, lv, yv(3), fl(d16))
            nc.vector.tensor_tensor(fl(sa[0]), ch(13), ch(12), OP.mult)
            nc.vector.tensor_tensor(dv(3), fl(d16), fl(sa[0]), OP.add)
            nc.vector.tensor_tensor(dv(4), ch(13), ch(13), OP.mult)
            nc.vector.tensor_tensor(dv(4), dv(4), fl(d16), OP.subtract)
            nc.vector.tensor_tensor(fl(sb[1]), ch(13), ch(14), OP.mult)
            nc.vector.tensor_tensor(dv(4), dv(4), fl(sb[1]), OP.add)
            nc.vector.tensor_tensor(dv(5), fl(d2), fl(d16), OP.subtract)
            nc.vector.tensor_tensor(fl(sb[1]), ch(12), ch(14), OP.mult)
            nc.vector.tensor_tensor(dv(5), dv(5), fl(sb[1]), OP.add)

            nc.scalar.activation(y2s[:], y2s[:], AF.Sqrt)

            ysrc = (0, 1, 2, 3, 3, 3)
            for i in range(6):
                E = EN[i % 2]
                E.tensor_scalar(dv(i), dv(i), 1e-30, None, OP.max)
                E.tensor_tensor(mv(i), yv(ysrc[i]), dv(i), OP.min)
                E.tensor_tensor(xv(i), yv(ysrc[i]), dv(i), OP.max)
            nc.vector.reciprocal(mxs[:], mxs[:])
            for i in range(6):
                EN[i % 2].tensor_tensor(mns[:, i, :], mns[:, i, :],
                                        mxs[:, i, :], OP.mult)
            nc.scalar.activation(mns[:], mns[:], AF.Arctan)
            for i in range(6):
                E = EN[i % 2]
                E.tensor_tensor(xv(i), yv(ysrc[i]), dv(i), OP.is_gt)
                E.tensor_scalar(dv(i), mv(i), -2.0, PIH, OP.mult, OP.add)
                E.tensor_tensor(dv(i), dv(i), xv(i), OP.mult)
                E.tensor_tensor(ch(ANGLE_CH[i]), mv(i), dv(i), OP.add)

            for bl in range(b0, b1):
                for g in range(NKK_G):
                    tp = pstp.tile([128, 128], F32, tag="tp")
                    nc.tensor.transpose(
                        tp[:],
                        F[:, bl, g * 4:(g + 1) * 4, :].rearrange(
                            "p k c -> p (k c)"),
                        ld["idn"][:])
                    nc.vector.tensor_copy(fcm[:, g, bl * 128:(bl + 1) * 128],
                                          tp[:])

            # L1 units for this half's two chunks (overlaps with phase S)
            for chunk in (2 * fh, 2 * fh + 1):
                for g in range(NKK_G):
                    for hw_ in range(2):
                        u = (chunk * NKK_G + g) * 2 + hw_
                        ps = psmm.tile([128, CW], F32, tag="ps")
                        lhs = ld["w1a"] if hw_ == 0 else ld["w1b"]
                        nc.tensor.matmul(
                            ps[:], lhs[:],
                            fcm[:, g, chunk * CW:(chunk + 1) * CW],
                            start=True, stop=True)
                        nc.scalar.activation(pre[:, u, :], ps[:], AF.Copy,
                                             accum_out=s1s[:, u:u + 1])
                        sq = sqp.tile([128, CW], BF16, tag="sq")
                        nc.scalar.activation(sq[:], ps[:], AF.Square,
                                             accum_out=s1q[:, u:u + 1])

    sfctx.close()  # free the feature tile before phase M allocations

    # ================= Phase M =================
    with ExitStack() as mctx:
        psst = mctx.enter_context(tc.tile_pool(name="psst", bufs=1,
                                               space="PSUM"))
        hp = mctx.enter_context(tc.tile_pool(name="hp", bufs=1))
        h3p = mctx.enter_context(tc.tile_pool(name="h3p", bufs=3))
        outp = mctx.enter_context(tc.tile_pool(name="outp", bufs=1))

        scale1 = stp.tile([128, 1], F32); bias1 = stp.tile([128, 1], F32)
        scale2 = stp.tile([128, 1], F32); bias2 = stp.tile([128, 1], F32)
        scale3 = stp.tile([96, 1], F32); bias3 = stp.tile([96, 1], F32)

        def finalize(s2, nelem, m1, expand, grep, brep, scale, bias, parts):
            gps = psst.tile([16, 2], F32, tag="gps")
            nc.tensor.matmul(gps[:], m1[:parts, :], s2[:], start=True,
                             stop=True)
            gsc = stp.tile([16, 2], F32, tag="gsc")
            nc.vector.tensor_copy(gsc[:], gps[:])
            inv_n = 1.0 / float(nelem)
            mg = stp.tile([16, 1], F32, tag="mg")
            vg = stp.tile([16, 1], F32, tag="vg")
            t2 = stp.tile([16, 1], F32, tag="t2")
            nc.vector.tensor_scalar(mg[:], gsc[:, 0:1], inv_n, None, OP.mult)
            nc.vector.tensor_scalar(vg[:], gsc[:, 1:2], inv_n, None, OP.mult)
            nc.vector.tensor_tensor(t2[:], mg[:], mg[:], OP.mult)
            nc.vector.tensor_tensor(vg[:], vg[:], t2[:], OP.subtract)
            nc.vector.tensor_scalar(vg[:], vg[:], GN_EPS, None, OP.add)
            nc.vector.reciprocal(vg[:], vg[:])
            nc.scalar.activation(vg[:], vg[:], AF.Sqrt)
            rm = stp.tile([16, 2], F32, tag="rm")
            nc.vector.tensor_copy(rm[:, 0:1], vg[:])
            nc.vector.tensor_copy(rm[:, 1:2], mg[:])
            eps_ = psst.tile([parts, 2], F32, tag="eps")
            nc.tensor.matmul(eps_[:], expand[:, :parts], rm[:], start=True,
                             stop=True)
            rexp = stp.tile([parts, 2], F32, tag=f"rexp{parts}")
            nc.vector.tensor_copy(rexp[:], eps_[:])
            nc.vector.tensor_tensor(scale[:parts, :], rexp[:, 0:1],
                                    grep[:parts, :], OP.mult)
            nc.vector.tensor_tensor(bias[:parts, :], rexp[:, 1:2],
                                    scale[:parts, :], OP.mult)
            nc.vector.tensor_tensor(bias[:parts, :], brep[:parts, :],
                                    bias[:parts, :], OP.subtract)

        def sums_finalize(ssum, ssq, nunits, nelem, m1, expand, grep, brep,
                          scale, bias, parts):
            s2 = stp.tile([parts, 2], F32, tag=f"s2{parts}")
            nc.vector.tensor_reduce(s2[:, 0:1], ssum[:parts, :nunits], AX.X,
                                    OP.add)
            nc.vector.tensor_reduce(s2[:, 1:2], ssq[:parts, :nunits], AX.X,
                                    OP.add)
            finalize(s2, nelem, m1, expand, grep, brep, scale, bias, parts)

        # (L1 matmuls/stats were emitted inside phase F, overlapped with S)
        sums_finalize(s1s, s1q, NU, 4 * NI * K, ld["m1_12"], ld["e_12"],
                      ld["g1rep"], ld["b1rep"], scale1, bias1, 128)
        if DBG == "s1":
            nc.gpsimd.dma_start(out_ap[:, :NU], s1s[:96, :])
            nc.gpsimd.dma_start(out_ap[:, NU:2 * NU], s1q[:96, :])
            nc.gpsimd.dma_start(out_ap[:, 2 * NU:2 * NU + 1], scale1[:96, :])
            nc.gpsimd.dma_start(out_ap[:, 2 * NU + 1:2 * NU + 2],
                                bias1[:96, :])
            return
        h1 = hp.tile([128, NU, CW], BF16, tag="h1")
        for j in range(4):
            nc.scalar.activation(h1[:, j * 10:(j + 1) * 10, :],
                                 pre[:, j * 10:(j + 1) * 10, :], AF.Relu,
                                 bias=bias1[:, :], scale=scale1[:, :])
        if DBG == "h1":
            nc.gpsimd.dma_start(out_ap[:, :NI],
                                h1[:96, 0:4, :].rearrange("p u w -> p (u w)"))
            return

        # ---- L2 ----
        s2s = stp.tile([128, NU], F32, tag="s2s")
        s2q = stp.tile([128, NU], F32, tag="s2q")
        pre2 = prep.tile([128, NU, CW], BF16, tag="pre")
        for u in range(NU):
            ps = psmm.tile([128, CW], F32, tag="ps")
            nc.tensor.matmul(ps[:], ld["w2bd"][:], h1[:, u, :], start=True,
                             stop=True)
            nc.scalar.activation(pre2[:, u, :], ps[:], AF.Copy,
                                 accum_out=s2s[:, u:u + 1])
            sq = sqp.tile([128, CW], BF16, tag="sq")
            nc.scalar.activation(sq[:], ps[:], AF.Square,
                                 accum_out=s2q[:, u:u + 1])
        sums_finalize(s2s, s2q, NU, 4 * NI * K, ld["m1_12"], ld["e_12"],
                      ld["g2rep"], ld["b2rep"], scale2, bias2, 128)
        h2 = hp.tile([128, NU, CW], BF16, tag="h2")
        for j in range(4):
            nc.scalar.activation(h2[:, j * 10:(j + 1) * 10, :],
                                 pre2[:, j * 10:(j + 1) * 10, :], AF.Relu,
                                 bias=bias2[:, :], scale=scale2[:, :])

        # ---- L3: bn_stats pass + recompute pass with norm+relu+max ----
        bn3 = stp.tile([96, NU3, 6], F32, tag="bn3")
        for u in range(NU):
            for kx in range(2):
                ps3 = psmm.tile([96, CW], F32, tag="ps")
                nc.tensor.matmul(ps3[:],
                                 ld["w3t"][kx * 64:(kx + 1) * 64, :],
                                 h2[kx * 64:(kx + 1) * 64, u, :],
                                 start=True, stop=True)
                u3 = u * 2 + kx
                nc.vector.bn_stats(bn3[:, u3, :], ps3[:])
        agg3 = stp.tile([96, 2], F32, tag="agg3")
        nc.vector.bn_aggr(agg3[:], bn3[:])
        s23 = stp.tile([96, 2], F32, tag="s23")
        npe3 = float(NU3 * CW)
        nc.vector.tensor_tensor(s23[:, 1:2], agg3[:, 0:1], agg3[:, 0:1],
                                OP.mult)
        nc.vector.tensor_tensor(s23[:, 1:2], s23[:, 1:2], agg3[:, 1:2],
                                OP.add)
        nc.vector.tensor_copy(s23[:, 0:1], agg3[:, 0:1])
        nc.vector.tensor_scalar(s23[:], s23[:], npe3, None, OP.mult)
        finalize(s23, 6 * NI * K, ld["m1_3"], ld["e_3"], ld["g3rep"],
                 ld["b3rep"], scale3, bias3, 96)

        outv = outp.tile([96, NI], F32, tag="outv")
        outg = outp.tile([96, NI], F32, tag="outg")
        nc.vector.memset(outv[:], 0.0)
        nc.vector.memset(outg[:], 0.0)
        for chunk in range(NCH):
            for g in range(NKK_G):
                for half in range(2):
                    u = (chunk * NKK_G + g) * 2 + half
                    for kx in range(2):
                        ps3 = psmm.tile([96, CW], F32, tag="ps")
                        nc.tensor.matmul(
                            ps3[:], ld["w3t"][kx * 64:(kx + 1) * 64, :],
                            h2[kx * 64:(kx + 1) * 64, u, :],
                            start=True, stop=True)
                        h3 = h3p.tile([96, CW], F32, tag="h3")
                        nc.scalar.activation(h3[:], ps3[:], AF.Relu,
                                             bias=bias3[:, :],
                                             scale=scale3[:, :])
                        acc = outv if (u * 2 + kx) % 2 == 0 else outg
                        nc.vector.tensor_tensor(
                            acc[:, chunk * CW:(chunk + 1) * CW],
                            acc[:, chunk * CW:(chunk + 1) * CW],
                            h3[:], OP.max)
        nc.vector.tensor_tensor(outv[:], outv[:], outg[:], OP.max)
        nc.gpsimd.dma_start(out_ap[:, :NI], outv[:])


# ======================= SPMD wrapper =======================
import concourse.bacc as bacc
from concourse.bass_utils import run_bass_kernel_spmd

_CACHE = {}


def _build_program():
    if "nc" in _CACHE:
        return _CACHE["nc"]
    nc = bacc.Bacc("TRN2", target_bir_lowering=False, debug=False,
                   num_devices=8)
    din = declare_inputs(nc)
    out_ap = nc.dram_tensor("out", [96, NI], F32, kind="ExternalOutput").ap()
    with tile.TileContext(nc) as tc:
        with ExitStack() as ctx:
            build(nc, tc, ctx, din, out_ap)
    nc.compile()
    _CACHE["nc"] = nc
    return nc


def kernel(**inputs):
    data = np.asarray(inputs["data"], dtype=np.float32)
    kk = int(np.asarray(inputs["k"]))
    assert kk == 20 and data.shape == (4, 6, 4096), (data.shape, kk)
    Wn = ["W1", "g1", "b1", "W2", "g2", "b2", "W3", "g3", "b3"]
    Wv = [np.asarray(inputs[n], dtype=np.float32) for n in Wn]
    nc = _build_program()
    in_maps = [host_prep(data[core // 2], core % 2, *Wv) for core in range(8)]
    res = run_bass_kernel_spmd(nc, in_maps, list(range(8)))
    out = np.empty((4, 96, N), np.float32)
    for b in range(4):
        for h in range(2):
            out[b][:, h * NI:(h + 1) * NI] = res.results[2 * b + h]["out"]
    return np.ascontiguousarray(out)
